# revision 6
# baseline (speedup 1.0000x reference)
"""MoE-routed attribute decoder kernel for 8x TRN2 NeuronCores.

Strategy
--------
The reference computes a dense (N,D)@(D,E*V) GEMM and then selects, per
voxel, the 16 outputs of its routed head.  Only the routed compute
(N*D*V MACs) is actually needed - 64x less than dense.

Host side (numpy, part of sharding):
  * compute per-voxel head id, stable-sort voxels by head,
  * pad each head's voxel list to a multiple of Q=512 with duplicate
    voxels (dup columns compute the same result; the scatter just
    rewrites the same value), so every head boundary lands on the
    512-column PSUM-chunk grid -> few, full-width matmuls,
  * cut the padded stream into 8 equal shards; piece boundaries are the
    union over cores of head-change offsets, so every core runs the
    *same* instruction stream (SPMD, one NEFF); the per-piece head
    weights are per-core *data* (packed weight inputs),
  * x shards are shipped pre-transposed (D on partitions) in fp16.

Device side (Bass/Tile):
  * stream X^T in [128 x 2048] tiles (4 k-tiles of the D=512
    contraction),
  * per 512-column chunk: K=1 matmul against a ones-row folds in the
    bias (start=True), then 4 accumulating K=128 matmuls,
  * PSUM -> SBUF copy (alternating DVE/ACT), per-superchunk DMA out.

Compute dtype fp16: absmax-relative error ~2.5e-4 vs the fp32 reference
(measured), half the HBM traffic of fp32 on a memory-bound stream.
Set BASS_KERNEL_MODE=fp32 for full precision.
"""

import os

import numpy as np

import concourse.bacc as bacc
import concourse.mybir as mybir
import concourse.tile as tile
from concourse.bass_utils import run_bass_kernel_spmd

N_CORES = 8
B, WD, HD, LD = 2, 32, 32, 32
D = 512
E = 64
V = 16
N = B * WD * HD * LD          # 65536 voxels
KT = D // 128                 # 4 k-tiles of the contraction
CHUNK = 512                   # one PSUM bank of fp32
SUPER = 2048                  # columns per streamed SBUF x-tile
Q = 512                       # per-head padding quantum (chunk-aligned)

_MODES = {
    "fp16": mybir.dt.float16,
    "bf16": mybir.dt.bfloat16,
    "fp32": mybir.dt.float32,
    "fp32r": mybir.dt.float32r,
}


def _np_dtype(mode):
    if mode == "bf16":
        import ml_dtypes

        return np.dtype(ml_dtypes.bfloat16)
    return np.dtype(np.float16 if mode == "fp16" else np.float32)


def _build_program(pieces, npieces, ncore, mode):
    """One SPMD program shared by all 8 cores.

    pieces: list of (slot, a, b) column ranges covering [0, ncore); all
    voxels in [a, b) on any given core share one head, whose weights sit
    in that core's packed weight input at slot `slot`.
    """
    dt_lo = _MODES[mode]

    # Bacc (not plain Bass): its compile() splits multi-wait sync onto
    # EventSemaphore instructions - TRN2 engine instructions have only
    # one hardware wait slot.
    nc = bacc.Bacc("TRN2", target_bir_lowering=False)
    xt = nc.dram_tensor("xt", [D, ncore], dt_lo, kind="ExternalInput")
    wt = nc.dram_tensor("wt", [128, KT * V * npieces], dt_lo, kind="ExternalInput")
    bt = nc.dram_tensor("bt", [1, V * npieces], dt_lo, kind="ExternalInput")
    yt = nc.dram_tensor("yt", [V, ncore], mybir.dt.float32, kind="ExternalOutput")

    with tile.TileContext(nc) as tc:
        with (
            tc.tile_pool(name="const", bufs=1) as constp,
            tc.tile_pool(name="xp", bufs=2) as xp,
            tc.tile_pool(name="yp", bufs=2) as yp,
            tc.tile_pool(name="psum", bufs=8, space="PSUM") as pp,
        ):
            wsb = constp.tile([128, KT * V * npieces], dt_lo)
            nc.sync.dma_start(wsb[:], wt[:])
            btsb = constp.tile([1, V * npieces], dt_lo)
            nc.sync.dma_start(btsb[:], bt[:])
            ones = constp.tile([1, CHUNK], dt_lo)
            nc.vector.memset(ones[:], 1.0)

            nchunk = 0
            for sc0 in range(0, ncore, SUPER):
                sc1 = min(sc0 + SUPER, ncore)
                wsc = sc1 - sc0
                xk = []
                for k in range(KT):
                    t = xp.tile([128, wsc], dt_lo, tag=f"xk{k}")
                    nc.sync.dma_start(t[:], xt[k * 128 : (k + 1) * 128, sc0:sc1])
                    xk.append(t)
                ysb = yp.tile([V, wsc], mybir.dt.float32, tag="ysb")
                for c0 in range(sc0, sc1, CHUNK):
                    c1 = min(c0 + CHUNK, sc1)
                    wch = c1 - c0
                    ps = pp.tile([V, wch], mybir.dt.float32, tag="ps")
                    for s, a, b in pieces:
                        a_, b_ = max(a, c0), min(b, c1)
                        if a_ >= b_:
                            continue
                        ra, rb = a_ - c0, b_ - c0
                        # Bias matmul first (start=True): K=1 against a
                        # ones-row adds b[head] into the accumulator.
                        nc.tensor.matmul(
                            ps[0:V, ra:rb],
                            btsb[0:1, s * V : (s + 1) * V],
                            ones[0:1, 0 : rb - ra],
                            start=True,
                            stop=False,
                        )
                        for k in range(KT):
                            nc.tensor.matmul(
                                ps[0:V, ra:rb],
                                wsb[:, (k * npieces + s) * V : (k * npieces + s + 1) * V],
                                xk[k][:, a_ - sc0 : b_ - sc0],
                                start=False,
                                stop=(k == KT - 1),
                            )
                    eng = nc.vector if nchunk % 2 == 0 else nc.scalar
                    if eng is nc.vector:
                        eng.tensor_copy(ysb[0:V, c0 - sc0 : c1 - sc0], ps[0:V, 0:wch])
                    else:
                        eng.copy(ysb[0:V, c0 - sc0 : c1 - sc0], ps[0:V, 0:wch])
                    nchunk += 1
                nc.sync.dma_start(yt[:, sc0:sc1], ysb[0:V, 0:wsc])
    # Run Bacc's compile passes; run_bass_via_pjrt does not finalize.
    nc.finalize()
    return nc


def kernel(block_type_grid, x, W_heads, b_heads, block2head):
    mode = os.environ.get("BASS_KERNEL_MODE", "fp16")
    dt_np = _np_dtype(mode)

    btg = np.asarray(block_type_grid).astype(np.int64).reshape(-1)
    b2h = np.asarray(block2head).astype(np.int64)
    xf = np.asarray(x, dtype=np.float32).reshape(N, D)
    Wh = np.asarray(W_heads, dtype=np.float32)
    bh = np.asarray(b_heads, dtype=np.float32)

    h = b2h[btg]                          # (N,) head per voxel
    order = np.argsort(h, kind="stable")  # sorted-by-head voxel stream
    hs = h[order]
    pfx = np.searchsorted(hs, np.arange(E + 1))

    # Pad each head to a multiple of Q with duplicate voxels, then pad the
    # total to a multiple of 8*Q so every shard boundary stays on the
    # Q-grid.
    counts = np.diff(pfx)
    n_pad = np.where(counts > 0, ((counts + Q - 1) // Q) * Q, 0)
    total = int(n_pad.sum())
    extra = (-total) % (N_CORES * Q)
    n_pad[int(np.argmax(n_pad))] += extra
    total += extra
    ncore = total // N_CORES

    chunks = []
    for e in range(E):
        ids = order[pfx[e] : pfx[e + 1]]
        if len(ids) == 0:
            continue
        pad = int(n_pad[e]) - len(ids)
        if pad:
            ids = np.concatenate([ids, np.repeat(ids[:1], pad)])
        chunks.append(ids)
    big = np.concatenate(chunks)          # (total,) padded voxel stream
    hbig = h[big]

    # Union of core-relative head boundaries (all on the Q grid).
    pp_ = np.cumsum(n_pad)
    offs = {0}
    for p in pp_:
        offs.add(int(p) % ncore)
    bounds = sorted(offs) + [ncore]
    pieces = []
    for i in range(len(bounds) - 1):
        if bounds[i + 1] > bounds[i]:
            pieces.append((len(pieces), bounds[i], bounds[i + 1]))
    npieces = len(pieces)

    WT = np.ascontiguousarray(Wh.transpose(0, 2, 1)).reshape(E, KT, 128, V)
    x_lo = xf.astype(dt_np)

    in_maps = []
    for c in range(N_CORES):
        sl = big[c * ncore : (c + 1) * ncore]
        xt_c = np.ascontiguousarray(x_lo[sl].T)        # (512, ncore)
        heads_c = hbig[c * ncore : (c + 1) * ncore]
        wt_c = np.zeros((128, KT * V * npieces), dt_np)
        bt_c = np.zeros((1, V * npieces), dt_np)
        for s, a, b in pieces:
            e = int(heads_c[a])
            for k in range(KT):
                wt_c[:, (k * npieces + s) * V : (k * npieces + s + 1) * V] = WT[e, k]
            bt_c[0, s * V : (s + 1) * V] = bh[e]
        in_maps.append({"xt": xt_c, "wt": wt_c, "bt": bt_c})

    nc = _build_program(pieces, npieces, ncore, mode)
    res = run_bass_kernel_spmd(nc, in_maps, core_ids=list(range(N_CORES)))

    out = np.zeros((N, V), np.float32)
    for c in range(N_CORES):
        out[big[c * ncore : (c + 1) * ncore]] = res.results[c]["yt"].T
    return out.reshape(B, WD, HD, LD, V)


# revision 9
# speedup vs baseline: 1.1712x; 1.1712x over previous
"""MoE-routed attribute decoder kernel for 8x TRN2 NeuronCores.

Strategy
--------
The reference computes a dense (N,D)@(D,E*V) GEMM and then selects, per
voxel, the 16 outputs of its routed head.  Only the routed compute
(N*D*V MACs) is actually needed - 64x less than dense.

Host side (numpy, part of sharding):
  * compute per-voxel head id, stable-sort voxels by head,
  * pad each head's voxel list to a multiple of Q=512 with duplicate
    voxels (dup columns compute the same result; the scatter just
    rewrites the same value), so every head boundary lands on the
    512-column PSUM-chunk grid -> few, full-width matmuls,
  * cut the padded stream into 8 equal shards; piece boundaries are the
    union over cores of head-change offsets (all on the 512 grid), so
    every core runs the *same* instruction stream (SPMD, one NEFF); the
    per-piece head weights/biases are per-core *data*,
  * x shards are shipped pre-transposed (D on partitions) in fp16.

Device side (Bass/Tile):
  * a short warm-up matmul burst on dummy zero tiles trips the PE HAM
    clock gate to 8/8 during the startup DMA window (otherwise the real
    stream runs at 1.2 GHz),
  * stream X^T in [128 x 2048] tiles (4 k-tiles of the D=512
    contraction); the first superchunk is one 512 chunk so compute
    starts early,
  * per 512-column chunk (one head per chunk by construction): 4
    accumulating K=128 matmuls; the PSUM->SBUF evacuation folds in the
    bias as a per-partition scalar add (DVE tensor_scalar_add / ACT
    activation-Identity, alternating), then a per-chunk DMA out.

Compute dtype fp16: absmax-relative error ~2.5e-4 vs the fp32 reference
(measured), half the HBM traffic of fp32 on a memory-bound stream.
Set BASS_KERNEL_MODE=fp32 for full precision.
"""

import os

import numpy as np

import concourse.bacc as bacc
import concourse.mybir as mybir
import concourse.tile as tile
from concourse.bass_utils import run_bass_kernel_spmd

N_CORES = 8
B, WD, HD, LD = 2, 32, 32, 32
D = 512
E = 64
V = 16
N = B * WD * HD * LD          # 65536 voxels
KT = D // 128                 # 4 k-tiles of the contraction
CHUNK = 512                   # one PSUM bank of fp32
SUPER = 4096                  # columns per streamed SBUF x-tile
Q = 512                       # per-head padding quantum (chunk-aligned)
WARMUP_MM = 9                 # dummy matmuls to warm the PE HAM gate

_MODES = {
    "fp16": mybir.dt.float16,
    "bf16": mybir.dt.bfloat16,
    "fp32": mybir.dt.float32,
    "fp32r": mybir.dt.float32r,
}


def _np_dtype(mode):
    if mode == "bf16":
        import ml_dtypes

        return np.dtype(ml_dtypes.bfloat16)
    return np.dtype(np.float16 if mode == "fp16" else np.float32)


def _build_program(pieces, npieces, ncore, mode):
    """One SPMD program shared by all 8 cores.

    pieces: list of (slot, a, b) column ranges covering [0, ncore), all
    on the 512 grid; all voxels in [a, b) on any given core share one
    head, whose weights sit in that core's packed weight input at slot
    `slot`.
    """
    dt_lo = _MODES[mode]

    # Bacc (not plain Bass): its compile() splits multi-wait sync onto
    # EventSemaphore instructions - TRN2 engine instructions have only
    # one hardware wait slot.
    nc = bacc.Bacc("TRN2", target_bir_lowering=False)
    xt = nc.dram_tensor("xt", [D, ncore], dt_lo, kind="ExternalInput")
    wt = nc.dram_tensor("wt", [128, KT * V * npieces], dt_lo, kind="ExternalInput")
    bt = nc.dram_tensor("bt", [V, npieces], mybir.dt.float32, kind="ExternalInput")
    yt = nc.dram_tensor("yt", [V, ncore], mybir.dt.float32, kind="ExternalOutput")

    # chunk -> owning piece slot (each 512 chunk is inside one piece)
    slot_of_chunk = {}
    for s, a, b in pieces:
        for c0 in range(a, b, CHUNK):
            slot_of_chunk[c0] = s

    with tile.TileContext(nc) as tc:
        with (
            tc.tile_pool(name="const", bufs=1) as constp,
            tc.tile_pool(name="xp", bufs=2) as xp,
            tc.tile_pool(name="yp", bufs=3) as yp,
            tc.tile_pool(name="psum", bufs=1, space="PSUM") as pp,
        ):
            # --- HAM warm-up: dummy matmuls on zeroed tiles ---
            wdum = constp.tile([128, V], dt_lo)
            nc.gpsimd.memset(wdum[:], 0.0)
            xdum = constp.tile([128, CHUNK], dt_lo)
            nc.gpsimd.memset(xdum[:], 0.0)
            pdum = pp.tile([V, CHUNK], mybir.dt.float32, tag="warm", bufs=1)
            for i in range(WARMUP_MM):
                nc.tensor.matmul(
                    pdum[0:V, :], wdum[:], xdum[:],
                    start=(i == 0), stop=(i == WARMUP_MM - 1),
                )

            wsb = constp.tile([128, KT * V * npieces], dt_lo)
            nc.sync.dma_start(wsb[:], wt[:])
            btsb = constp.tile([V, npieces], mybir.dt.float32)
            nc.sync.dma_start(btsb[:], bt[:])

            # Variable superchunks: small first one so compute starts early.
            sbounds = [0, min(CHUNK, ncore)]
            while sbounds[-1] < ncore:
                sbounds.append(min(sbounds[-1] + SUPER, ncore))

            for sc0, sc1 in zip(sbounds[:-1], sbounds[1:]):
                wsc = sc1 - sc0
                xk = []
                for k in range(KT):
                    t = xp.tile([128, wsc], dt_lo, tag=f"xk{k}")
                    nc.sync.dma_start(t[:], xt[k * 128 : (k + 1) * 128, sc0:sc1])
                    xk.append(t)
                ysb = yp.tile([V, wsc], mybir.dt.float32, tag="ysb")
                for c0 in range(sc0, sc1, CHUNK):
                    c1 = min(c0 + CHUNK, sc1)
                    wch = c1 - c0
                    s = slot_of_chunk[c0]
                    ps = pp.tile([V, wch], mybir.dt.float32, tag="ps", bufs=6)
                    for k in range(KT):
                        nc.tensor.matmul(
                            ps[0:V, 0:wch],
                            wsb[:, (k * npieces + s) * V : (k * npieces + s + 1) * V],
                            xk[k][:, c0 - sc0 : c1 - sc0],
                            start=(k == 0),
                            stop=(k == KT - 1),
                        )
                    # PSUM -> SBUF evacuation + bias add (per-partition
                    # scalar) on DVE.
                    nc.vector.tensor_scalar_add(
                        ysb[0:V, c0 - sc0 : c1 - sc0], ps[0:V, 0:wch],
                        btsb[0:V, s : s + 1],
                    )
                # Output DMA on the ACT HWDGE ring - keeps the Sync ring
                # free for input streaming (each dma_start occupies its
                # ring ~0.6us regardless of size).
                nc.scalar.dma_start(yt[:, sc0:sc1], ysb[0:V, 0:wsc])
    # Run Bacc's compile passes; run_bass_via_pjrt does not finalize.
    nc.finalize()
    return nc


def kernel(block_type_grid, x, W_heads, b_heads, block2head):
    mode = os.environ.get("BASS_KERNEL_MODE", "fp16")
    dt_np = _np_dtype(mode)

    btg = np.asarray(block_type_grid).astype(np.int64).reshape(-1)
    b2h = np.asarray(block2head).astype(np.int64)
    xf = np.asarray(x, dtype=np.float32).reshape(N, D)
    Wh = np.asarray(W_heads, dtype=np.float32)
    bh = np.asarray(b_heads, dtype=np.float32)

    h = b2h[btg]                          # (N,) head per voxel
    order = np.argsort(h, kind="stable")  # sorted-by-head voxel stream
    hs = h[order]
    pfx = np.searchsorted(hs, np.arange(E + 1))

    # Pad each head to a multiple of Q with duplicate voxels, then pad the
    # total to a multiple of 8*Q so every shard boundary stays on the
    # Q-grid.
    counts = np.diff(pfx)
    n_pad = np.where(counts > 0, ((counts + Q - 1) // Q) * Q, 0)
    total = int(n_pad.sum())
    extra = (-total) % (N_CORES * Q)
    n_pad[int(np.argmax(n_pad))] += extra
    total += extra
    ncore = total // N_CORES

    chunks = []
    for e in range(E):
        ids = order[pfx[e] : pfx[e + 1]]
        if len(ids) == 0:
            continue
        pad = int(n_pad[e]) - len(ids)
        if pad:
            ids = np.concatenate([ids, np.repeat(ids[:1], pad)])
        chunks.append(ids)
    big = np.concatenate(chunks)          # (total,) padded voxel stream
    hbig = h[big]

    # Union of core-relative head boundaries (all on the Q grid).
    pp_ = np.cumsum(n_pad)
    offs = {0}
    for p in pp_:
        offs.add(int(p) % ncore)
    bounds = sorted(offs) + [ncore]
    pieces = []
    for i in range(len(bounds) - 1):
        if bounds[i + 1] > bounds[i]:
            pieces.append((len(pieces), bounds[i], bounds[i + 1]))
    npieces = len(pieces)

    WT = np.ascontiguousarray(Wh.transpose(0, 2, 1)).reshape(E, KT, 128, V)
    x_lo = xf.astype(dt_np)

    in_maps = []
    for c in range(N_CORES):
        sl = big[c * ncore : (c + 1) * ncore]
        xt_c = np.ascontiguousarray(x_lo[sl].T)        # (512, ncore)
        heads_c = hbig[c * ncore : (c + 1) * ncore]
        wt_c = np.zeros((128, KT * V * npieces), dt_np)
        bt_c = np.zeros((V, npieces), np.float32)
        for s, a, b in pieces:
            e = int(heads_c[a])
            for k in range(KT):
                wt_c[:, (k * npieces + s) * V : (k * npieces + s + 1) * V] = WT[e, k]
            bt_c[:, s] = bh[e]
        in_maps.append({"xt": xt_c, "wt": wt_c, "bt": bt_c})

    nc = _build_program(pieces, npieces, ncore, mode)
    res = run_bass_kernel_spmd(nc, in_maps, core_ids=list(range(N_CORES)))

    out = np.zeros((N, V), np.float32)
    for c in range(N_CORES):
        out[big[c * ncore : (c + 1) * ncore]] = res.results[c]["yt"].T
    return out.reshape(B, WD, HD, LD, V)


# revision 11
# speedup vs baseline: 1.1806x; 1.0081x over previous
"""MoE-routed attribute decoder kernel for 8x TRN2 NeuronCores.

Strategy
--------
The reference computes a dense (N,D)@(D,E*V) GEMM and then selects, per
voxel, the 16 outputs of its routed head.  Only the routed compute
(N*D*V MACs) is actually needed - 64x less than dense.

Host side (numpy, part of sharding):
  * compute per-voxel head id, stable-sort voxels by head,
  * pad each head's voxel list to a multiple of Q=512 with duplicate
    voxels (dup columns compute the same result; the scatter just
    rewrites the same value), so every head boundary lands on the
    512-column PSUM-chunk grid -> few, full-width matmuls,
  * cut the padded stream into 8 equal shards; piece boundaries are the
    union over cores of head-change offsets (all on the 512 grid), so
    every core runs the *same* instruction stream (SPMD, one NEFF); the
    per-piece head weights/biases are per-core *data*,
  * x shards are shipped pre-transposed (D on partitions) in fp16.

Device side (Bass/Tile):
  * a short warm-up matmul burst on dummy zero tiles trips the PE HAM
    clock gate to 8/8 during the startup DMA window (otherwise the real
    stream runs at 1.2 GHz),
  * stream X^T in [128 x 2048] tiles (4 k-tiles of the D=512
    contraction); the first superchunk is one 512 chunk so compute
    starts early,
  * per 512-column chunk (one head per chunk by construction): 4
    accumulating K=128 matmuls; the PSUM->SBUF evacuation folds in the
    bias as a per-partition scalar add (DVE tensor_scalar_add / ACT
    activation-Identity, alternating), then a per-chunk DMA out.

Compute dtype fp16: absmax-relative error ~2.5e-4 vs the fp32 reference
(measured), half the HBM traffic of fp32 on a memory-bound stream.
Set BASS_KERNEL_MODE=fp32 for full precision.
"""

import os

import numpy as np

import concourse.bacc as bacc
import concourse.mybir as mybir
import concourse.tile as tile
from concourse.bass_utils import run_bass_kernel_spmd

N_CORES = 8
B, WD, HD, LD = 2, 32, 32, 32
D = 512
E = 64
V = 16
N = B * WD * HD * LD          # 65536 voxels
KT = D // 128                 # 4 k-tiles of the contraction
CHUNK = 512                   # one PSUM bank of fp32
SUPER = 4096                  # columns per streamed SBUF x-tile
Q = 512                       # per-head padding quantum (chunk-aligned)
WARMUP_MM = 9                 # dummy matmuls to warm the PE HAM gate

_MODES = {
    "fp16": mybir.dt.float16,
    "bf16": mybir.dt.bfloat16,
    "fp32": mybir.dt.float32,
    "fp32r": mybir.dt.float32r,
}


def _np_dtype(mode):
    if mode == "bf16":
        import ml_dtypes

        return np.dtype(ml_dtypes.bfloat16)
    return np.dtype(np.float16 if mode == "fp16" else np.float32)


def _build_program(pieces, npieces, ncore, mode):
    """One SPMD program shared by all 8 cores.

    pieces: list of (slot, a, b) column ranges covering [0, ncore), all
    on the 512 grid; all voxels in [a, b) on any given core share one
    head, whose weights sit in that core's packed weight input at slot
    `slot`.
    """
    dt_lo = _MODES[mode]

    # Bacc (not plain Bass): its compile() splits multi-wait sync onto
    # EventSemaphore instructions - TRN2 engine instructions have only
    # one hardware wait slot.
    nc = bacc.Bacc("TRN2", target_bir_lowering=False)
    xt = nc.dram_tensor("xt", [D, ncore], dt_lo, kind="ExternalInput")
    wt = nc.dram_tensor("wt", [128, KT * V * npieces], dt_lo, kind="ExternalInput")
    bt = nc.dram_tensor("bt", [V, npieces], mybir.dt.float32, kind="ExternalInput")
    yt = nc.dram_tensor("yt", [V, ncore], mybir.dt.float32, kind="ExternalOutput")

    # chunk -> owning piece slot (each 512 chunk is inside one piece)
    slot_of_chunk = {}
    for s, a, b in pieces:
        for c0 in range(a, b, CHUNK):
            slot_of_chunk[c0] = s

    with tile.TileContext(nc) as tc:
        with (
            tc.tile_pool(name="const", bufs=1) as constp,
            tc.tile_pool(name="xp", bufs=2) as xp,
            tc.tile_pool(name="yp", bufs=3) as yp,
            tc.tile_pool(name="psum", bufs=1, space="PSUM") as pp,
        ):
            # --- HAM warm-up: dummy matmuls on zeroed tiles ---
            wdum = constp.tile([128, V], dt_lo)
            nc.gpsimd.memset(wdum[:], 0.0)
            xdum = constp.tile([128, CHUNK], dt_lo)
            nc.gpsimd.memset(xdum[:], 0.0)
            pdum = pp.tile([V, CHUNK], mybir.dt.float32, tag="ps", bufs=8)
            for i in range(WARMUP_MM):
                nc.tensor.matmul(
                    pdum[0:V, :], wdum[:], xdum[:],
                    start=(i == 0), stop=(i == WARMUP_MM - 1),
                )

            wsb = constp.tile([128, KT * V * npieces], dt_lo)
            nc.sync.dma_start(wsb[:], wt[:])
            btsb = constp.tile([V, npieces], mybir.dt.float32)
            nc.sync.dma_start(btsb[:], bt[:])

            # Variable superchunks: small first one so compute starts early.
            sbounds = [0, min(CHUNK, ncore)]
            while sbounds[-1] < ncore:
                sbounds.append(min(sbounds[-1] + SUPER, ncore))

            for sc0, sc1 in zip(sbounds[:-1], sbounds[1:]):
                wsc = sc1 - sc0
                xk = []
                for k in range(KT):
                    t = xp.tile([128, wsc], dt_lo, tag=f"xk{k}")
                    nc.sync.dma_start(t[:], xt[k * 128 : (k + 1) * 128, sc0:sc1])
                    xk.append(t)
                ysb = yp.tile([V, wsc], mybir.dt.float32, tag="ysb")
                for c0 in range(sc0, sc1, CHUNK):
                    c1 = min(c0 + CHUNK, sc1)
                    wch = c1 - c0
                    s = slot_of_chunk[c0]
                    ps = pp.tile([V, wch], mybir.dt.float32, tag="ps", bufs=8)
                    for k in range(KT):
                        nc.tensor.matmul(
                            ps[0:V, 0:wch],
                            wsb[:, (k * npieces + s) * V : (k * npieces + s + 1) * V],
                            xk[k][:, c0 - sc0 : c1 - sc0],
                            start=(k == 0),
                            stop=(k == KT - 1),
                        )
                    # PSUM -> SBUF evacuation + bias add (per-partition
                    # scalar), alternating DVE / ACT so the slot frees
                    # promptly even when the scheduler bunches evacs.
                    dst = ysb[0:V, c0 - sc0 : c1 - sc0]
                    bias_ap = btsb[0:V, s : s + 1]
                    if (c0 // CHUNK) % 2 == 0:
                        nc.vector.tensor_scalar_add(dst, ps[0:V, 0:wch], bias_ap)
                    else:
                        nc.scalar.add(dst, ps[0:V, 0:wch], bias_ap)
                nc.sync.dma_start(yt[:, sc0:sc1], ysb[0:V, 0:wsc])
    # Run Bacc's compile passes; run_bass_via_pjrt does not finalize.
    nc.finalize()
    return nc


def kernel(block_type_grid, x, W_heads, b_heads, block2head):
    mode = os.environ.get("BASS_KERNEL_MODE", "fp16")
    dt_np = _np_dtype(mode)

    btg = np.asarray(block_type_grid).astype(np.int64).reshape(-1)
    b2h = np.asarray(block2head).astype(np.int64)
    xf = np.asarray(x, dtype=np.float32).reshape(N, D)
    Wh = np.asarray(W_heads, dtype=np.float32)
    bh = np.asarray(b_heads, dtype=np.float32)

    h = b2h[btg]                          # (N,) head per voxel
    order = np.argsort(h, kind="stable")  # sorted-by-head voxel stream
    hs = h[order]
    pfx = np.searchsorted(hs, np.arange(E + 1))

    # Pad each head to a multiple of Q with duplicate voxels, then pad the
    # total to a multiple of 8*Q so every shard boundary stays on the
    # Q-grid.
    counts = np.diff(pfx)
    n_pad = np.where(counts > 0, ((counts + Q - 1) // Q) * Q, 0)
    total = int(n_pad.sum())
    extra = (-total) % (N_CORES * Q)
    n_pad[int(np.argmax(n_pad))] += extra
    total += extra
    ncore = total // N_CORES

    chunks = []
    for e in range(E):
        ids = order[pfx[e] : pfx[e + 1]]
        if len(ids) == 0:
            continue
        pad = int(n_pad[e]) - len(ids)
        if pad:
            ids = np.concatenate([ids, np.repeat(ids[:1], pad)])
        chunks.append(ids)
    big = np.concatenate(chunks)          # (total,) padded voxel stream
    hbig = h[big]

    # Union of core-relative head boundaries (all on the Q grid).
    pp_ = np.cumsum(n_pad)
    offs = {0}
    for p in pp_:
        offs.add(int(p) % ncore)
    bounds = sorted(offs) + [ncore]
    pieces = []
    for i in range(len(bounds) - 1):
        if bounds[i + 1] > bounds[i]:
            pieces.append((len(pieces), bounds[i], bounds[i + 1]))
    npieces = len(pieces)

    WT = np.ascontiguousarray(Wh.transpose(0, 2, 1)).reshape(E, KT, 128, V)
    x_lo = xf.astype(dt_np)

    in_maps = []
    for c in range(N_CORES):
        sl = big[c * ncore : (c + 1) * ncore]
        xt_c = np.ascontiguousarray(x_lo[sl].T)        # (512, ncore)
        heads_c = hbig[c * ncore : (c + 1) * ncore]
        wt_c = np.zeros((128, KT * V * npieces), dt_np)
        bt_c = np.zeros((V, npieces), np.float32)
        for s, a, b in pieces:
            e = int(heads_c[a])
            for k in range(KT):
                wt_c[:, (k * npieces + s) * V : (k * npieces + s + 1) * V] = WT[e, k]
            bt_c[:, s] = bh[e]
        in_maps.append({"xt": xt_c, "wt": wt_c, "bt": bt_c})

    nc = _build_program(pieces, npieces, ncore, mode)
    res = run_bass_kernel_spmd(nc, in_maps, core_ids=list(range(N_CORES)))

    out = np.zeros((N, V), np.float32)
    for c in range(N_CORES):
        out[big[c * ncore : (c + 1) * ncore]] = res.results[c]["yt"].T
    return out.reshape(B, WD, HD, LD, V)


# revision 14
# speedup vs baseline: 1.2058x; 1.0213x over previous
"""MoE-routed attribute decoder kernel for 8x TRN2 NeuronCores.

Strategy
--------
The reference computes a dense (N,D)@(D,E*V) GEMM and then selects, per
voxel, the 16 outputs of its routed head.  Only the routed compute
(N*D*V MACs) is actually needed - 64x less than dense.

Host side (numpy, part of sharding):
  * compute per-voxel head id, stable-sort voxels by head,
  * pad each head's voxel list to a multiple of Q=512 with duplicate
    voxels (dup columns compute the same result; the scatter just
    rewrites the same value), so every head boundary lands on the
    512-column PSUM-chunk grid -> few, full-width matmuls,
  * cut the padded stream into 8 equal shards; piece boundaries are the
    union over cores of head-change offsets (all on the 512 grid), so
    every core runs the *same* instruction stream (SPMD, one NEFF); the
    per-piece head weights/biases are per-core *data*,
  * x shards are shipped pre-transposed (D on partitions) in fp16.

Device side (Bass/Tile):
  * a short warm-up matmul burst on dummy zero tiles trips the PE HAM
    clock gate to 8/8 during the startup DMA window (otherwise the real
    stream runs at 1.2 GHz),
  * stream X^T in [128 x 2048] tiles (4 k-tiles of the D=512
    contraction); the first superchunk is one 512 chunk so compute
    starts early,
  * per 512-column chunk (one head per chunk by construction): 4
    accumulating K=128 matmuls; the PSUM->SBUF evacuation folds in the
    bias as a per-partition scalar add (DVE tensor_scalar_add / ACT
    activation-Identity, alternating), then a per-chunk DMA out.

Compute dtype fp16: absmax-relative error ~2.5e-4 vs the fp32 reference
(measured), half the HBM traffic of fp32 on a memory-bound stream.
Set BASS_KERNEL_MODE=fp32 for full precision.
"""

import os

import numpy as np

import concourse.bacc as bacc
import concourse.mybir as mybir
import concourse.tile as tile
from concourse.bass_utils import run_bass_kernel_spmd

N_CORES = 8
B, WD, HD, LD = 2, 32, 32, 32
D = 512
E = 64
V = 16
N = B * WD * HD * LD          # 65536 voxels
KT = D // 128                 # 4 k-tiles of the contraction
CHUNK = 512                   # one PSUM bank of fp32
SUPER = 4096                  # columns per streamed SBUF x-tile
Q = 512                       # per-head padding quantum (chunk-aligned)
WARMUP_MM = 9                 # dummy matmuls to warm the PE HAM gate

_MODES = {
    "fp16": mybir.dt.float16,
    "bf16": mybir.dt.bfloat16,
    "fp32": mybir.dt.float32,
    "fp32r": mybir.dt.float32r,
}


def _np_dtype(mode):
    if mode == "bf16":
        import ml_dtypes

        return np.dtype(ml_dtypes.bfloat16)
    return np.dtype(np.float16 if mode == "fp16" else np.float32)


def _build_program(pieces, npieces, ncore, mode):
    """One SPMD program shared by all 8 cores.

    pieces: list of (slot, a, b) column ranges covering [0, ncore), all
    on the 512 grid; all voxels in [a, b) on any given core share one
    head, whose weights sit in that core's packed weight input at slot
    `slot`.
    """
    dt_lo = _MODES[mode]

    # Bacc (not plain Bass): its compile() splits multi-wait sync onto
    # EventSemaphore instructions - TRN2 engine instructions have only
    # one hardware wait slot.
    nc = bacc.Bacc("TRN2", target_bir_lowering=False)
    xt = nc.dram_tensor("xt", [D, ncore], dt_lo, kind="ExternalInput")
    wt = nc.dram_tensor("wt", [128, KT * V * npieces], dt_lo, kind="ExternalInput")
    bt = nc.dram_tensor("bt", [V, npieces], mybir.dt.float32, kind="ExternalInput")
    yt = nc.dram_tensor("yt", [V, ncore], mybir.dt.float32, kind="ExternalOutput")

    # chunk -> owning piece slot (each 512 chunk is inside one piece)
    slot_of_chunk = {}
    for s, a, b in pieces:
        for c0 in range(a, b, CHUNK):
            slot_of_chunk[c0] = s

    with tile.TileContext(nc) as tc:
        with (
            tc.tile_pool(name="const", bufs=1) as constp,
            tc.tile_pool(name="xp", bufs=2) as xp,
            tc.tile_pool(name="yp", bufs=3) as yp,
            tc.tile_pool(name="psum", bufs=1, space="PSUM") as pp,
        ):
            # --- HAM warm-up: dummy matmuls on zeroed tiles ---
            wdum = constp.tile([128, V], dt_lo)
            nc.gpsimd.memset(wdum[:], 0.0)
            xdum = constp.tile([128, CHUNK], dt_lo)
            nc.gpsimd.memset(xdum[:], 0.0)
            pdum = pp.tile([V, CHUNK], mybir.dt.float32, tag="ps", bufs=8)
            for i in range(WARMUP_MM):
                nc.tensor.matmul(
                    pdum[0:V, :], wdum[:], xdum[:],
                    start=(i == 0), stop=(i == WARMUP_MM - 1),
                )

            wsb = constp.tile([128, KT * V * npieces], dt_lo)
            nc.sync.dma_start(wsb[:], wt[:])
            btsb = constp.tile([V, npieces], mybir.dt.float32)
            nc.sync.dma_start(btsb[:], bt[:])

            # Variable superchunks: small first one so compute starts
            # early, small last one so the drain tail is short.
            sbounds = [0, min(CHUNK, ncore)]
            while sbounds[-1] < ncore:
                sbounds.append(min(sbounds[-1] + SUPER, ncore))
            if sbounds[-1] - sbounds[-2] > 2 * CHUNK:
                sbounds.insert(-1, sbounds[-1] - CHUNK)

            for sc0, sc1 in zip(sbounds[:-1], sbounds[1:]):
                wsc = sc1 - sc0
                xk = []
                for k in range(KT):
                    t = xp.tile([128, wsc], dt_lo, tag=f"xk{k}", bufs=3)
                    nc.sync.dma_start(t[:], xt[k * 128 : (k + 1) * 128, sc0:sc1])
                    xk.append(t)
                ysb = yp.tile([V, wsc], mybir.dt.float32, tag="ysb")
                for c0 in range(sc0, sc1, CHUNK):
                    c1 = min(c0 + CHUNK, sc1)
                    wch = c1 - c0
                    s = slot_of_chunk[c0]
                    ps = pp.tile([V, wch], mybir.dt.float32, tag="ps", bufs=8)
                    for k in range(KT):
                        nc.tensor.matmul(
                            ps[0:V, 0:wch],
                            wsb[:, (k * npieces + s) * V : (k * npieces + s + 1) * V],
                            xk[k][:, c0 - sc0 : c1 - sc0],
                            start=(k == 0),
                            stop=(k == KT - 1),
                        )
                    # PSUM -> SBUF evacuation + bias add (per-partition
                    # scalar), alternating DVE / ACT so the slot frees
                    # promptly even when the scheduler bunches evacs.
                    dst = ysb[0:V, c0 - sc0 : c1 - sc0]
                    bias_ap = btsb[0:V, s : s + 1]
                    if (c0 // CHUNK) % 2 == 0:
                        nc.vector.tensor_scalar_add(dst, ps[0:V, 0:wch], bias_ap)
                    else:
                        nc.scalar.add(dst, ps[0:V, 0:wch], bias_ap)
                # Output DMA on the GPSIMD (SWDGE) ring: it waits on the
                # last evac, and on the Sync ring it would block the next
                # superchunk's input DMA issue (FIFO per ring).
                nc.gpsimd.dma_start(yt[:, sc0:sc1], ysb[0:V, 0:wsc])
    # Run Bacc's compile passes; run_bass_via_pjrt does not finalize.
    nc.finalize()
    return nc


def kernel(block_type_grid, x, W_heads, b_heads, block2head):
    mode = os.environ.get("BASS_KERNEL_MODE", "fp16")
    dt_np = _np_dtype(mode)

    btg = np.asarray(block_type_grid).astype(np.int64).reshape(-1)
    b2h = np.asarray(block2head).astype(np.int64)
    xf = np.asarray(x, dtype=np.float32).reshape(N, D)
    Wh = np.asarray(W_heads, dtype=np.float32)
    bh = np.asarray(b_heads, dtype=np.float32)

    h = b2h[btg]                          # (N,) head per voxel
    order = np.argsort(h, kind="stable")  # sorted-by-head voxel stream
    hs = h[order]
    pfx = np.searchsorted(hs, np.arange(E + 1))

    # Pad each head to a multiple of Q with duplicate voxels, then pad the
    # total to a multiple of 8*Q so every shard boundary stays on the
    # Q-grid.
    counts = np.diff(pfx)
    n_pad = np.where(counts > 0, ((counts + Q - 1) // Q) * Q, 0)
    total = int(n_pad.sum())
    extra = (-total) % (N_CORES * Q)
    n_pad[int(np.argmax(n_pad))] += extra
    total += extra
    ncore = total // N_CORES

    chunks = []
    for e in range(E):
        ids = order[pfx[e] : pfx[e + 1]]
        if len(ids) == 0:
            continue
        pad = int(n_pad[e]) - len(ids)
        if pad:
            ids = np.concatenate([ids, np.repeat(ids[:1], pad)])
        chunks.append(ids)
    big = np.concatenate(chunks)          # (total,) padded voxel stream
    hbig = h[big]

    # Union of core-relative head boundaries (all on the Q grid).
    pp_ = np.cumsum(n_pad)
    offs = {0}
    for p in pp_:
        offs.add(int(p) % ncore)
    bounds = sorted(offs) + [ncore]
    pieces = []
    for i in range(len(bounds) - 1):
        if bounds[i + 1] > bounds[i]:
            pieces.append((len(pieces), bounds[i], bounds[i + 1]))
    npieces = len(pieces)

    WT = np.ascontiguousarray(Wh.transpose(0, 2, 1)).reshape(E, KT, 128, V)
    x_lo = xf.astype(dt_np)

    in_maps = []
    for c in range(N_CORES):
        sl = big[c * ncore : (c + 1) * ncore]
        xt_c = np.ascontiguousarray(x_lo[sl].T)        # (512, ncore)
        heads_c = hbig[c * ncore : (c + 1) * ncore]
        wt_c = np.zeros((128, KT * V * npieces), dt_np)
        bt_c = np.zeros((V, npieces), np.float32)
        for s, a, b in pieces:
            e = int(heads_c[a])
            for k in range(KT):
                wt_c[:, (k * npieces + s) * V : (k * npieces + s + 1) * V] = WT[e, k]
            bt_c[:, s] = bh[e]
        in_maps.append({"xt": xt_c, "wt": wt_c, "bt": bt_c})

    nc = _build_program(pieces, npieces, ncore, mode)
    res = run_bass_kernel_spmd(nc, in_maps, core_ids=list(range(N_CORES)))

    out = np.zeros((N, V), np.float32)
    for c in range(N_CORES):
        out[big[c * ncore : (c + 1) * ncore]] = res.results[c]["yt"].T
    return out.reshape(B, WD, HD, LD, V)


# revision 18
# speedup vs baseline: 1.2735x; 1.0562x over previous
"""MoE-routed attribute decoder kernel for 8x TRN2 NeuronCores.

Strategy
--------
The reference computes a dense (N,D)@(D,E*V) GEMM and then selects, per
voxel, the 16 outputs of its routed head.  Only the routed compute
(N*D*V MACs) is actually needed - 64x less than dense.

Host side (numpy, part of sharding):
  * compute per-voxel head id, stable-sort voxels by head,
  * pad each head's voxel list to a multiple of Q=512 with duplicate
    voxels (dup columns compute the same result; the scatter just
    rewrites the same value), so every head boundary lands on the
    512-column PSUM-chunk grid -> few, full-width matmuls,
  * cut the padded stream into 8 equal shards; piece boundaries are the
    union over cores of head-change offsets (all on the 512 grid), so
    every core runs the *same* instruction stream (SPMD, one NEFF); the
    per-piece head weights/biases are per-core *data*,
  * x shards are shipped pre-transposed (D on partitions) in fp16.

Device side (Bass/Tile):
  * a short warm-up matmul burst on dummy zero tiles trips the PE HAM
    clock gate to 8/8 during the startup DMA window (otherwise the real
    stream runs at 1.2 GHz),
  * stream X^T in [128 x 2048] tiles (4 k-tiles of the D=512
    contraction); the first superchunk is one 512 chunk so compute
    starts early,
  * per 512-column chunk (one head per chunk by construction): 4
    accumulating K=128 matmuls; the PSUM->SBUF evacuation folds in the
    bias as a per-partition scalar add (DVE tensor_scalar_add / ACT
    activation-Identity, alternating), then a per-chunk DMA out.

Compute dtype fp16: absmax-relative error ~2.5e-4 vs the fp32 reference
(measured), half the HBM traffic of fp32 on a memory-bound stream.
Set BASS_KERNEL_MODE=fp32 for full precision.
"""

import os

import numpy as np

import concourse.bacc as bacc
import concourse.mybir as mybir
import concourse.tile as tile
from concourse.bass_utils import run_bass_kernel_spmd

N_CORES = 8
B, WD, HD, LD = 2, 32, 32, 32
D = 512
E = 64
V = 16
N = B * WD * HD * LD          # 65536 voxels
KT = D // 128                 # 4 k-tiles of the contraction
CHUNK = 512                   # one PSUM bank of fp32
SUPER = 2048                  # columns per streamed SBUF x-tile
Q = 512                       # per-head padding quantum (chunk-aligned)
WARMUP_MM = 9                 # dummy matmuls to warm the PE HAM gate

_MODES = {
    "fp16": mybir.dt.float16,
    "bf16": mybir.dt.bfloat16,
    "fp32": mybir.dt.float32,
    "fp32r": mybir.dt.float32r,
}


def _np_dtype(mode):
    if mode == "bf16":
        import ml_dtypes

        return np.dtype(ml_dtypes.bfloat16)
    return np.dtype(np.float16 if mode == "fp16" else np.float32)


def _build_program(pieces, npieces, ncore, mode):
    """One SPMD program shared by all 8 cores.

    pieces: list of (slot, a, b) column ranges covering [0, ncore), all
    on the 512 grid; all voxels in [a, b) on any given core share one
    head, whose weights sit in that core's packed weight input at slot
    `slot`.
    """
    dt_lo = _MODES[mode]

    # Bacc (not plain Bass): its compile() splits multi-wait sync onto
    # EventSemaphore instructions - TRN2 engine instructions have only
    # one hardware wait slot.
    nc = bacc.Bacc("TRN2", target_bir_lowering=False)
    xt = nc.dram_tensor("xt", [D, ncore], dt_lo, kind="ExternalInput")
    wt = nc.dram_tensor("wt", [128, KT * V * npieces], dt_lo, kind="ExternalInput")
    bt = nc.dram_tensor("bt", [V, npieces], mybir.dt.float32, kind="ExternalInput")
    yt = nc.dram_tensor("yt", [V, ncore], mybir.dt.float32, kind="ExternalOutput")

    # chunk -> owning piece slot (each 512 chunk is inside one piece)
    slot_of_chunk = {}
    for s, a, b in pieces:
        for c0 in range(a, b, CHUNK):
            slot_of_chunk[c0] = s

    with tile.TileContext(nc) as tc:
        with (
            tc.tile_pool(name="const", bufs=1) as constp,
            tc.tile_pool(name="xp", bufs=2) as xp,
            tc.tile_pool(name="yp", bufs=3) as yp,
            tc.tile_pool(name="psum", bufs=1, space="PSUM") as pp,
        ):
            # --- HAM warm-up: dummy matmuls on zeroed tiles ---
            wdum = constp.tile([128, V], dt_lo)
            nc.gpsimd.memset(wdum[:], 0.0)
            xdum = constp.tile([128, CHUNK], dt_lo)
            nc.gpsimd.memset(xdum[:], 0.0)
            pdum = pp.tile([V, CHUNK], mybir.dt.float32, tag="ps", bufs=8)
            for i in range(WARMUP_MM):
                nc.tensor.matmul(
                    pdum[0:V, :], wdum[:], xdum[:],
                    start=(i == 0), stop=(i == WARMUP_MM - 1),
                )

            wsb = constp.tile([128, KT * V * npieces], dt_lo)
            nc.sync.dma_start(wsb[:], wt[:])
            btsb = constp.tile([V, npieces], mybir.dt.float32)
            nc.sync.dma_start(btsb[:], bt[:])

            # Variable superchunks: ramp up (compute starts early, DMA
            # arrival granularity stays ahead), small last one so the
            # drain tail is short.
            sbounds = [0]
            step = CHUNK
            while sbounds[-1] < ncore:
                sbounds.append(min(sbounds[-1] + step, ncore))
                step = min(step * 2, SUPER)
            if sbounds[-1] - sbounds[-2] > 2 * CHUNK:
                sbounds.insert(-1, sbounds[-1] - CHUNK)

            for sc0, sc1 in zip(sbounds[:-1], sbounds[1:]):
                wsc = sc1 - sc0
                xk = []
                for k in range(KT):
                    t = xp.tile([128, wsc], dt_lo, tag=f"xk{k}", bufs=3)
                    nc.sync.dma_start(t[:], xt[k * 128 : (k + 1) * 128, sc0:sc1])
                    xk.append(t)
                ysb = yp.tile([V, wsc], mybir.dt.float32, tag="ysb")
                cbounds = list(range(sc0, sc1, CHUNK)) + [sc1]
                pst = []
                for a, b in zip(cbounds[:-1], cbounds[1:]):
                    ps = pp.tile([V, b - a], mybir.dt.float32, tag="ps", bufs=8, name="ps")
                    pst.append(ps)
                # k-outer: one LDWEIGHTS per (k, piece) instead of per
                # (k, chunk) - consecutive chunks of a piece reuse the
                # stationary operand.
                for k in range(KT):
                    for ci, (c0, c1) in enumerate(zip(cbounds[:-1], cbounds[1:])):
                        s = slot_of_chunk[c0]
                        nc.tensor.matmul(
                            pst[ci][0:V, 0 : c1 - c0],
                            wsb[:, (k * npieces + s) * V : (k * npieces + s + 1) * V],
                            xk[k][:, c0 - sc0 : c1 - sc0],
                            start=(k == 0),
                            stop=(k == KT - 1),
                            skip_group_check=True,
                        )
                for ci, (c0, c1) in enumerate(zip(cbounds[:-1], cbounds[1:])):
                    s = slot_of_chunk[c0]
                    # PSUM -> SBUF evacuation + bias add (per-partition
                    # scalar), alternating DVE / ACT.
                    dst = ysb[0:V, c0 - sc0 : c1 - sc0]
                    bias_ap = btsb[0:V, s : s + 1]
                    if (c0 // CHUNK) % 2 == 0:
                        nc.vector.tensor_scalar_add(dst, pst[ci][0:V, 0 : c1 - c0], bias_ap)
                    else:
                        nc.scalar.add(dst, pst[ci][0:V, 0 : c1 - c0], bias_ap)
                # Output DMA on the GPSIMD (SWDGE) ring: it waits on the
                # last evac, and on the Sync ring it would block the next
                # superchunk's input DMA issue (FIFO per ring).
                nc.gpsimd.dma_start(yt[:, sc0:sc1], ysb[0:V, 0:wsc])
    # Run Bacc's compile passes; run_bass_via_pjrt does not finalize.
    nc.finalize()
    return nc


def kernel(block_type_grid, x, W_heads, b_heads, block2head):
    mode = os.environ.get("BASS_KERNEL_MODE", "fp16")
    dt_np = _np_dtype(mode)

    btg = np.asarray(block_type_grid).astype(np.int64).reshape(-1)
    b2h = np.asarray(block2head).astype(np.int64)
    xf = np.asarray(x, dtype=np.float32).reshape(N, D)
    Wh = np.asarray(W_heads, dtype=np.float32)
    bh = np.asarray(b_heads, dtype=np.float32)

    h = b2h[btg]                          # (N,) head per voxel
    order = np.argsort(h, kind="stable")  # sorted-by-head voxel stream
    hs = h[order]
    pfx = np.searchsorted(hs, np.arange(E + 1))

    # Pad each head to a multiple of Q with duplicate voxels, then pad the
    # total to a multiple of 8*Q so every shard boundary stays on the
    # Q-grid.
    counts = np.diff(pfx)
    n_pad = np.where(counts > 0, ((counts + Q - 1) // Q) * Q, 0)
    total = int(n_pad.sum())
    extra = (-total) % (N_CORES * Q)
    n_pad[int(np.argmax(n_pad))] += extra
    total += extra
    ncore = total // N_CORES

    chunks = []
    for e in range(E):
        ids = order[pfx[e] : pfx[e + 1]]
        if len(ids) == 0:
            continue
        pad = int(n_pad[e]) - len(ids)
        if pad:
            ids = np.concatenate([ids, np.repeat(ids[:1], pad)])
        chunks.append(ids)
    big = np.concatenate(chunks)          # (total,) padded voxel stream
    hbig = h[big]

    # Union of core-relative head boundaries (all on the Q grid).
    pp_ = np.cumsum(n_pad)
    offs = {0}
    for p in pp_:
        offs.add(int(p) % ncore)
    bounds = sorted(offs) + [ncore]
    pieces = []
    for i in range(len(bounds) - 1):
        if bounds[i + 1] > bounds[i]:
            pieces.append((len(pieces), bounds[i], bounds[i + 1]))
    npieces = len(pieces)

    WT = np.ascontiguousarray(Wh.transpose(0, 2, 1)).reshape(E, KT, 128, V)
    x_lo = xf.astype(dt_np)

    in_maps = []
    for c in range(N_CORES):
        sl = big[c * ncore : (c + 1) * ncore]
        xt_c = np.ascontiguousarray(x_lo[sl].T)        # (512, ncore)
        heads_c = hbig[c * ncore : (c + 1) * ncore]
        wt_c = np.zeros((128, KT * V * npieces), dt_np)
        bt_c = np.zeros((V, npieces), np.float32)
        for s, a, b in pieces:
            e = int(heads_c[a])
            for k in range(KT):
                wt_c[:, (k * npieces + s) * V : (k * npieces + s + 1) * V] = WT[e, k]
            bt_c[:, s] = bh[e]
        in_maps.append({"xt": xt_c, "wt": wt_c, "bt": bt_c})

    nc = _build_program(pieces, npieces, ncore, mode)
    res = run_bass_kernel_spmd(nc, in_maps, core_ids=list(range(N_CORES)))

    out = np.zeros((N, V), np.float32)
    for c in range(N_CORES):
        out[big[c * ncore : (c + 1) * ncore]] = res.results[c]["yt"].T
    return out.reshape(B, WD, HD, LD, V)
